# revision 7
# baseline (speedup 1.0000x reference)
"""BertInsertion loss kernel for 8 Trainium2 NeuronCores.

Strategy: pure data parallelism over the batch (64 rows -> 8 rows/core).
Each core, entirely on device:
  1. Finds the C=16 marked positions per row from the 0/1 sot mask
     (cumsum via tensor_tensor_scan + count-compare, no sort).
  2. Ragged-gathers the 16 D=1024 vectors per row from HBM with an
     indirect DMA (only ~0.5 MB read instead of the 16 MB shard).
  3. Normalizes, computes cosine sims vs the group's first vector via a
     broadcast matmul + fused multiply-reduce.
  4. Computes per-row cross-entropy (log-sum-exp; no max shift needed
     since |cos sim| <= 1) and the argmax prediction.
Host only reshards inputs, supplies small input-independent constant
matrices, concatenates the 8-row partials, and takes the masked mean.
"""

import numpy as np

import concourse.bass as bass
import concourse.bacc as bacc
import concourse.tile as tile
from concourse import mybir
from concourse.bass_utils import run_bass_kernel_spmd

B, S, D, C = 64, 512, 1024, 16
NCORES = 8
BL = B // NCORES  # batch rows per core
P = BL * C        # 128 gathered vectors per core = full partition dim
EPS = 1e-6
NEG = -40.0       # below any cosine sim; exp(-40) ~ 4e-18 is invisible in f32

F32 = mybir.dt.float32
I32 = mybir.dt.int32
Op = mybir.AluOpType
Act = mybir.ActivationFunctionType

# constant-pack column layout
CA_W = C + BL + BL + 1 + 1          # [128, 34]
CB_W = P + C + 1                    # [8, 145]


def _make_consts():
    p = np.arange(P)
    b = np.arange(BL)
    onehotPT = (p[:, None] % C == np.arange(C)[None, :]).astype(np.float32)
    ET = (p[:, None] // C == b[None, :]).astype(np.float32)
    selSp = (p[:, None] == C * b[None, :]).astype(np.float32)
    rvals = (p % C + 1).astype(np.float32)[:, None]
    offv = ((p // C) * S).astype(np.float32)[:, None]
    ca = np.ascontiguousarray(
        np.concatenate([onehotPT, ET, selSp, rvals, offv], axis=1))
    E = (np.arange(P)[None, :] // C == b[:, None]).astype(np.float32)
    it16 = np.broadcast_to(np.arange(C, dtype=np.float32), (BL, C))
    spioff = (b * S).astype(np.float32)[:, None]
    cb = np.ascontiguousarray(np.concatenate([E, it16, spioff], axis=1))
    return ca, cb


def _build():
    nc = bacc.Bacc("TRN2", target_bir_lowering=False, debug=False)

    seq = nc.dram_tensor("seq", [BL * S, D], F32, kind="ExternalInput").ap()
    mask = nc.dram_tensor("mask", [BL, S], I32, kind="ExternalInput").ap()
    lab = nc.dram_tensor("lab", [BL, 1], I32, kind="ExternalInput").ap()
    ca_d = nc.dram_tensor("ca", [P, CA_W], F32, kind="ExternalInput").ap()
    cb_d = nc.dram_tensor("cb", [BL, CB_W], F32, kind="ExternalInput").ap()
    ce_out = nc.dram_tensor("ce", [BL, 1], F32, kind="ExternalOutput").ap()
    pred_out = nc.dram_tensor("pred", [BL, 1], I32, kind="ExternalOutput").ap()

    with tile.TileContext(nc) as tc:
        import contextlib

        with contextlib.ExitStack() as ctx:
            pool = ctx.enter_context(tc.tile_pool(name="sb", bufs=1))
            psum = ctx.enter_context(tc.tile_pool(name="ps", bufs=1, space="PSUM"))

            # ---------------- constants via DMA ----------------
            ca = pool.tile([P, CA_W], F32)
            nc.sync.dma_start(out=ca[:], in_=ca_d)
            cb = pool.tile([BL, CB_W], F32)
            nc.sync.dma_start(out=cb[:], in_=cb_d)
            onehotPT = ca[:, 0:C]
            ET = ca[:, C:C + BL]
            selSp = ca[:, C + BL:C + 2 * BL]
            rvals = ca[:, C + 2 * BL:C + 2 * BL + 1]
            offv = ca[:, C + 2 * BL + 1:C + 2 * BL + 2]
            E = cb[:, 0:P]
            it16 = cb[:, P:P + C]
            spioff = cb[:, P + C:P + C + 1]

            # ---------------- stage A: mask -> gather indices ----------------
            m_i = pool.tile([BL, S], I32)
            nc.sync.dma_start(out=m_i[:], in_=mask)
            m_f = pool.tile([BL, S], F32)
            nc.vector.tensor_copy(out=m_f[:], in_=m_i[:])
            zeros = pool.tile([BL, S], F32)
            nc.vector.memset(zeros[:], 0.0)
            cum = pool.tile([BL, S], F32)  # inclusive cumsum of the mask
            nc.vector.tensor_tensor_scan(out=cum[:], data0=m_f[:], data1=zeros[:],
                                         initial=0.0, op0=Op.add, op1=Op.add)

            # broadcast each row's cumsum to its 16 partitions
            cumB = psum.tile([P, S], F32)
            nc.tensor.matmul(out=cumB[:], lhsT=E, rhs=cum[:],
                             start=True, stop=True)
            # idx(rank r, row b) = #{s : cum[b,s] < r}
            ind = pool.tile([P, S], F32)
            nc.vector.tensor_scalar(out=ind[:], in0=cumB[:], scalar1=rvals,
                                    scalar2=None, op0=Op.is_lt)
            idx_f = pool.tile([P, 1], F32)
            nc.vector.tensor_reduce(out=idx_f[:], in_=ind[:],
                                    axis=mybir.AxisListType.X, op=Op.add)
            idx_g = pool.tile([P, 1], F32)
            nc.vector.tensor_tensor(out=idx_g[:], in0=idx_f[:], in1=offv,
                                    op=Op.add)
            idx_i = pool.tile([P, 1], I32)
            nc.vector.tensor_copy(out=idx_i[:], in_=idx_g[:])

            # speaker (first marked position) index per row, in [BL,1] layout
            spcnt = pool.tile([BL, S], F32)
            nc.vector.tensor_scalar(out=spcnt[:], in0=cum[:], scalar1=1.0,
                                    scalar2=None, op0=Op.is_lt)
            spidx_f = pool.tile([BL, 1], F32)
            nc.vector.tensor_reduce(out=spidx_f[:], in_=spcnt[:],
                                    axis=mybir.AxisListType.X, op=Op.add)
            spidx_g = pool.tile([BL, 1], F32)
            nc.vector.tensor_tensor(out=spidx_g[:], in0=spidx_f[:],
                                    in1=spioff, op=Op.add)
            spidx_i = pool.tile([BL, 1], I32)
            nc.vector.tensor_copy(out=spidx_i[:], in_=spidx_g[:])

            # ---------------- stage B: ragged gathers ----------------
            G = pool.tile([P, D], F32)
            nc.gpsimd.indirect_dma_start(
                out=G[:], out_offset=None, in_=seq,
                in_offset=bass.IndirectOffsetOnAxis(ap=idx_i[:, :1], axis=0))
            Spk = pool.tile([BL, D], F32)
            nc.gpsimd.indirect_dma_start(
                out=Spk[:], out_offset=None, in_=seq,
                in_offset=bass.IndirectOffsetOnAxis(ap=spidx_i[:, :1], axis=0))

            # ---------------- stage C: norms + cosine sims ----------------
            sq = pool.tile([P, D], F32)
            n2 = pool.tile([P, 1], F32)
            nc.scalar.activation(out=sq[:], in_=G[:], func=Act.Square,
                                 accum_out=n2[:])
            nrm = pool.tile([P, 1], F32)
            nc.scalar.activation(out=nrm[:], in_=n2[:], func=Act.Sqrt)
            nc.vector.tensor_scalar_max(out=nrm[:], in0=nrm[:], scalar1=EPS)
            rnorm = pool.tile([P, 1], F32)
            nc.vector.reciprocal(out=rnorm[:], in_=nrm[:])

            # per-group speaker 1/norm, broadcast back to all 128 partitions
            rsp = psum.tile([BL, 1], F32)
            nc.tensor.matmul(out=rsp[:], lhsT=selSp, rhs=rnorm[:],
                             start=True, stop=True)
            rsp_sb = pool.tile([BL, 1], F32)
            nc.vector.tensor_copy(out=rsp_sb[:], in_=rsp[:])
            rspB = psum.tile([P, 1], F32)
            nc.tensor.matmul(out=rspB[:], lhsT=E, rhs=rsp_sb[:],
                             start=True, stop=True)
            rspB_sb = pool.tile([P, 1], F32)
            nc.vector.tensor_copy(out=rspB_sb[:], in_=rspB[:])

            # broadcast raw speaker vectors to their 16 partitions
            SpB = psum.tile([P, D], F32)
            for h in range(2):
                cs = slice(h * 512, (h + 1) * 512)
                nc.tensor.matmul(out=SpB[:, cs], lhsT=E, rhs=Spk[:, cs],
                                 start=True, stop=True)
            dotscr = pool.tile([P, D], F32)
            dotraw = pool.tile([P, 1], F32)
            for h in range(2):
                cs = slice(h * 512, (h + 1) * 512)
                nc.vector.tensor_tensor(out=dotscr[:, cs], in0=G[:, cs],
                                        in1=SpB[:, cs], op=Op.mult)
            nc.vector.tensor_reduce(out=dotraw[:], in_=dotscr[:],
                                    axis=mybir.AxisListType.X, op=Op.add)
            simv = pool.tile([P, 1], F32)
            nc.vector.tensor_scalar(out=simv[:], in0=dotraw[:],
                                    scalar1=rnorm[:], scalar2=rspB_sb[:],
                                    op0=Op.mult, op1=Op.mult)

            # ---------------- stage D: per-row CE + argmax ----------------
            simSpread = pool.tile([P, C], F32)
            nc.vector.tensor_scalar_mul(out=simSpread[:], in0=onehotPT,
                                        scalar1=simv[:])
            simGrid = psum.tile([BL, C], F32)
            nc.tensor.matmul(out=simGrid[:], lhsT=ET, rhs=simSpread[:],
                             start=True, stop=True)
            sg = pool.tile([BL, C], F32)
            nc.vector.tensor_copy(out=sg[:], in_=simGrid[:])
            nc.vector.memset(sg[:, 0:1], NEG)

            expv = pool.tile([BL, C], F32)
            sumexp = pool.tile([BL, 1], F32)
            nc.scalar.activation(out=expv[:], in_=sg[:], func=Act.Exp,
                                 accum_out=sumexp[:])
            lse = pool.tile([BL, 1], F32)
            nc.scalar.activation(out=lse[:], in_=sumexp[:], func=Act.Ln)

            lab_i = pool.tile([BL, 1], I32)
            nc.sync.dma_start(out=lab_i[:], in_=lab)
            labp1 = pool.tile([BL, 1], F32)
            nc.vector.tensor_copy(out=labp1[:], in_=lab_i[:])
            nc.vector.tensor_scalar_add(out=labp1[:], in0=labp1[:], scalar1=1.0)
            onehotF = pool.tile([BL, C], F32)
            nc.vector.tensor_scalar(out=onehotF[:], in0=it16,
                                    scalar1=labp1[:], scalar2=None,
                                    op0=Op.is_equal)
            selscr = pool.tile([BL, C], F32)
            selv = pool.tile([BL, 1], F32)
            nc.vector.tensor_tensor(out=selscr[:], in0=sg[:],
                                    in1=onehotF[:], op=Op.mult)
            nc.vector.tensor_reduce(out=selv[:], in_=selscr[:],
                                    axis=mybir.AxisListType.X, op=Op.add)
            cet = pool.tile([BL, 1], F32)
            nc.vector.tensor_tensor(out=cet[:], in0=lse[:], in1=selv[:],
                                    op=Op.subtract)
            nc.sync.dma_start(out=ce_out, in_=cet[:])

            mx = pool.tile([BL, 8], F32)
            mi = pool.tile([BL, 8], mybir.dt.uint32)
            nc.vector.max_with_indices(out_max=mx[:], out_indices=mi[:], in_=sg[:])
            pred_f = pool.tile([BL, 1], F32)
            nc.vector.tensor_copy(out=pred_f[:], in_=mi[:, 0:1])
            nc.vector.tensor_scalar_add(out=pred_f[:], in0=pred_f[:],
                                        scalar1=-1.0)
            pred_i = pool.tile([BL, 1], I32)
            nc.vector.tensor_copy(out=pred_i[:], in_=pred_f[:])
            nc.sync.dma_start(out=pred_out, in_=pred_i[:])

    nc.compile()
    return nc


_NC = None
_CONSTS = None


def _get_nc():
    global _NC
    if _NC is None:
        _NC = _build()
    return _NC


def _get_consts():
    global _CONSTS
    if _CONSTS is None:
        _CONSTS = _make_consts()
    return _CONSTS


def kernel(sequence_output, sot_positions, labels):
    seq = np.ascontiguousarray(np.asarray(sequence_output, dtype=np.float32))
    mask = np.ascontiguousarray(np.asarray(sot_positions, dtype=np.int32))
    lab = np.ascontiguousarray(np.asarray(labels, dtype=np.int32))
    ca, cb = _get_consts()

    in_maps = []
    for i in range(NCORES):
        r = slice(i * BL, (i + 1) * BL)
        in_maps.append({
            "seq": seq[r].reshape(BL * S, D),
            "mask": mask[r],
            "lab": lab[r].reshape(BL, 1),
            "ca": ca,
            "cb": cb,
        })

    res = run_bass_kernel_spmd(_get_nc(), in_maps, core_ids=list(range(NCORES)))
    ce = np.concatenate([np.asarray(r["ce"])[:, 0] for r in res.results])
    pred = np.concatenate([np.asarray(r["pred"])[:, 0] for r in res.results])

    valid = lab >= 0
    n_valid = np.float32(valid.sum())
    loss = np.float32(
        np.sum(np.where(valid, ce, np.float32(0.0)), dtype=np.float32)
        / max(n_valid, np.float32(1.0)))
    return loss, pred.astype(np.int32), lab


# revision 9
# speedup vs baseline: 1.0519x; 1.0519x over previous
"""BertInsertion loss kernel for 8 Trainium2 NeuronCores.

Strategy: pure data parallelism over the batch (64 rows -> 8 rows/core).
Each core, entirely on device:
  1. Finds the C=16 marked positions per row from the 0/1 sot mask
     (bf16 cumsum scan + fused count-compare, no sort).
  2. Ragged-gathers the 16 D=1024 vectors per row from HBM with an
     indirect DMA (~0.5 MB read instead of the 16 MB shard).
  3. Cosine sims vs the group's first vector via broadcast matmuls and
     a multiply-reduce; 1/norm computed as exp(-0.5*ln(x2)) so only the
     Exp/Ln/Square ACT tables are needed (pre-warmed at kernel start).
  4. Per-row cross-entropy (log-sum-exp; |cos sim| <= 1 so no max
     shift) and argmax prediction, packed into one [8,2] output.
Host only reshards inputs, supplies input-independent constants, and
takes the masked mean over the 64 per-row CE values.
"""

import numpy as np
import ml_dtypes

import concourse.bass as bass
import concourse.bacc as bacc
import concourse.tile as tile
from concourse import mybir
from concourse.bass_utils import run_bass_kernel_spmd

B, S, D, C = 64, 512, 1024, 16
NCORES = 8
BL = B // NCORES  # batch rows per core
P = BL * C        # 128 gathered vectors per core = full partition dim
RMAX = 1e6        # 1/EPS, clamp for 1/norm (matches x/max(||x||,1e-6))
NEG = -40.0       # below any cosine sim; exp(-40) ~ 4e-18, invisible in f32

F32 = mybir.dt.float32
BF16 = mybir.dt.bfloat16
I32 = mybir.dt.int32
U32 = mybir.dt.uint32
Op = mybir.AluOpType
Act = mybir.ActivationFunctionType

CA_W = C + BL + BL + 1 + 1          # [128, 34] f32 pack
CB_W = P + C + 1                    # [8, 145] f32 pack


def _make_consts():
    p = np.arange(P)
    b = np.arange(BL)
    onehotPT = (p[:, None] % C == np.arange(C)[None, :]).astype(np.float32)
    ET = (p[:, None] // C == b[None, :]).astype(np.float32)
    selSp = (p[:, None] == C * b[None, :]).astype(np.float32)
    rvals = (p % C + 1).astype(np.float32)[:, None]
    offv = ((p // C) * S).astype(np.float32)[:, None]
    ca = np.ascontiguousarray(
        np.concatenate([onehotPT, ET, selSp, rvals, offv], axis=1))
    E = (np.arange(P)[None, :] // C == b[:, None]).astype(np.float32)
    it16 = np.broadcast_to(np.arange(C, dtype=np.float32), (BL, C)).copy()
    spioff = (b * S).astype(np.float32)[:, None]
    cb = np.ascontiguousarray(np.concatenate([E, it16, spioff], axis=1))
    ebf = np.ascontiguousarray(E.astype(ml_dtypes.bfloat16))
    return ca, cb, ebf


def _build():
    nc = bacc.Bacc("TRN2", target_bir_lowering=False, debug=False)

    seq = nc.dram_tensor("seq", [BL * S, D], F32, kind="ExternalInput").ap()
    maskf = nc.dram_tensor("maskf", [BL, S], BF16, kind="ExternalInput").ap()
    labp1 = nc.dram_tensor("labp1", [BL, 1], F32, kind="ExternalInput").ap()
    ca_d = nc.dram_tensor("ca", [P, CA_W], F32, kind="ExternalInput").ap()
    cb_d = nc.dram_tensor("cb", [BL, CB_W], F32, kind="ExternalInput").ap()
    ebf_d = nc.dram_tensor("ebf", [BL, P], BF16, kind="ExternalInput").ap()
    out_d = nc.dram_tensor("out", [BL, 2], F32, kind="ExternalOutput").ap()

    with tile.TileContext(nc) as tc:
        import contextlib

        with contextlib.ExitStack() as ctx:
            pool = ctx.enter_context(tc.tile_pool(name="sb", bufs=1))
            psum = ctx.enter_context(tc.tile_pool(name="ps", bufs=1, space="PSUM"))

            # -------- input DMAs: mask first on the sync HWDGE queue -----
            m_bf = pool.tile([BL, S], BF16)
            nc.sync.dma_start(out=m_bf[:], in_=maskf)
            lp1 = pool.tile([BL, 1], F32)
            nc.sync.dma_start(out=lp1[:], in_=labp1)
            # consts on the scalar HWDGE queue (parallel FIFO)
            ca = pool.tile([P, CA_W], F32)
            nc.scalar.dma_start(out=ca[:], in_=ca_d)
            cb = pool.tile([BL, CB_W], F32)
            nc.scalar.dma_start(out=cb[:], in_=cb_d)
            ebf = pool.tile([BL, P], BF16)
            nc.scalar.dma_start(out=ebf[:], in_=ebf_d)

            onehotPT = ca[:, 0:C]
            ET = ca[:, C:C + BL]
            selSp = ca[:, C + BL:C + 2 * BL]
            rvals = ca[:, C + 2 * BL:C + 2 * BL + 1]
            offv = ca[:, C + 2 * BL + 1:C + 2 * BL + 2]
            E = cb[:, 0:P]
            it16 = cb[:, P:P + C]
            spioff = cb[:, P + C:P + C + 1]

            # -------- pre-warm the ACT function tables --------------------
            w = pool.tile([1, 1], F32)
            nc.vector.memset(w[:], 1.0)
            wo = pool.tile([1, 3], F32)
            nc.scalar.activation(out=wo[:, 0:1], in_=w[:], func=Act.Square)
            nc.scalar.activation(out=wo[:, 1:2], in_=w[:], func=Act.Exp)
            nc.scalar.activation(out=wo[:, 2:3], in_=w[:], func=Act.Ln)

            # -------- stage A: mask -> gather indices ---------------------
            zeros = pool.tile([BL, S], BF16)
            nc.vector.memset(zeros[:], 0.0)
            cum = pool.tile([BL, S], BF16)  # inclusive cumsum (values <= 16)
            nc.vector.tensor_tensor_scan(out=cum[:], data0=m_bf[:],
                                         data1=zeros[:], initial=0.0,
                                         op0=Op.add, op1=Op.add)

            # speaker (first marked position): ready before the matmul path
            spcnt = pool.tile([BL, S], F32)
            spidx_f = pool.tile([BL, 1], F32)
            nc.vector.tensor_scalar(out=spcnt[:], in0=cum[:], scalar1=1.0,
                                    scalar2=None, op0=Op.is_lt, op1=Op.add,
                                    accum_out=spidx_f[:])
            spidx_i = pool.tile([BL, 1], I32)
            nc.vector.tensor_scalar(out=spidx_i[:], in0=spidx_f[:],
                                    scalar1=spioff, scalar2=None, op0=Op.add)
            Spk = pool.tile([BL, D], F32)
            nc.gpsimd.indirect_dma_start(
                out=Spk[:], out_offset=None, in_=seq,
                in_offset=bass.IndirectOffsetOnAxis(ap=spidx_i[:, :1], axis=0))

            # broadcast each row's cumsum to its 16 partitions (bf16 PE)
            cumB = psum.tile([P, S], F32)
            nc.tensor.matmul(out=cumB[:], lhsT=ebf[:], rhs=cum[:],
                             start=True, stop=True)
            ind = pool.tile([P, S], F32)
            idx_f = pool.tile([P, 1], F32)
            nc.vector.tensor_scalar(out=ind[:], in0=cumB[:], scalar1=rvals,
                                    scalar2=None, op0=Op.is_lt, op1=Op.add,
                                    accum_out=idx_f[:])
            idx_i = pool.tile([P, 1], I32)
            nc.vector.tensor_scalar(out=idx_i[:], in0=idx_f[:], scalar1=offv,
                                    scalar2=None, op0=Op.add)
            G = pool.tile([P, D], F32)
            nc.gpsimd.indirect_dma_start(
                out=G[:], out_offset=None, in_=seq,
                in_offset=bass.IndirectOffsetOnAxis(ap=idx_i[:, :1], axis=0))

            # -------- stage C: norms + cosine sims ------------------------
            # speaker 1/norm in [8,1] layout, then broadcast via PE
            sqsp = pool.tile([BL, D], F32)
            n2sp = pool.tile([BL, 1], F32)
            nc.scalar.activation(out=sqsp[:], in_=Spk[:], func=Act.Square,
                                 accum_out=n2sp[:])
            rlsp = pool.tile([BL, 1], F32)
            nc.scalar.activation(out=rlsp[:], in_=n2sp[:], func=Act.Ln)
            rs_sp_u = pool.tile([BL, 1], F32)
            nc.scalar.activation(out=rs_sp_u[:], in_=rlsp[:], func=Act.Exp,
                                 scale=-0.5)
            rs_sp = pool.tile([BL, 1], F32)
            nc.vector.tensor_scalar_min(out=rs_sp[:], in0=rs_sp_u[:],
                                        scalar1=RMAX)
            rspB = psum.tile([P, 1], F32)
            nc.tensor.matmul(out=rspB[:], lhsT=E, rhs=rs_sp[:],
                             start=True, stop=True)
            rspB_sb = pool.tile([P, 1], F32)
            nc.vector.tensor_copy(out=rspB_sb[:], in_=rspB[:])

            # broadcast raw speaker vectors to their 16 partitions
            SpB = psum.tile([P, D], F32)
            for h in range(2):
                cs = slice(h * 512, (h + 1) * 512)
                nc.tensor.matmul(out=SpB[:, cs], lhsT=E, rhs=Spk[:, cs],
                                 start=True, stop=True)

            # main 1/norms
            sq = pool.tile([P, D], F32)
            n2 = pool.tile([P, 1], F32)
            nc.scalar.activation(out=sq[:], in_=G[:], func=Act.Square,
                                 accum_out=n2[:])
            rl = pool.tile([P, 1], F32)
            nc.scalar.activation(out=rl[:], in_=n2[:], func=Act.Ln)
            rs_u = pool.tile([P, 1], F32)
            nc.scalar.activation(out=rs_u[:], in_=rl[:], func=Act.Exp,
                                 scale=-0.5)
            rs = pool.tile([P, 1], F32)
            nc.vector.tensor_scalar_min(out=rs[:], in0=rs_u[:], scalar1=RMAX)

            # dot products
            dotscr = pool.tile([P, D], F32)
            dotraw = pool.tile([P, 1], F32)
            for h in range(2):
                cs = slice(h * 512, (h + 1) * 512)
                nc.vector.tensor_tensor(out=dotscr[:, cs], in0=G[:, cs],
                                        in1=SpB[:, cs], op=Op.mult)
            nc.vector.tensor_reduce(out=dotraw[:], in_=dotscr[:],
                                    axis=mybir.AxisListType.X, op=Op.add)
            simv = pool.tile([P, 1], F32)
            nc.vector.tensor_scalar(out=simv[:], in0=dotraw[:],
                                    scalar1=rs[:], scalar2=rspB_sb[:],
                                    op0=Op.mult, op1=Op.mult)

            # -------- stage D: per-row CE + argmax ------------------------
            simSpread = pool.tile([P, C], F32)
            nc.vector.tensor_scalar_mul(out=simSpread[:], in0=onehotPT,
                                        scalar1=simv[:])
            simGrid = psum.tile([BL, C], F32)
            nc.tensor.matmul(out=simGrid[:], lhsT=ET, rhs=simSpread[:],
                             start=True, stop=True)
            sg = pool.tile([BL, C], F32)
            nc.vector.tensor_copy(out=sg[:], in_=simGrid[:])
            nc.vector.memset(sg[:, 0:1], NEG)

            expv = pool.tile([BL, C], F32)
            sumexp = pool.tile([BL, 1], F32)
            nc.scalar.activation(out=expv[:], in_=sg[:], func=Act.Exp,
                                 accum_out=sumexp[:])
            lse = pool.tile([BL, 1], F32)
            nc.scalar.activation(out=lse[:], in_=sumexp[:], func=Act.Ln)

            onehotF = pool.tile([BL, C], F32)
            nc.vector.tensor_scalar(out=onehotF[:], in0=it16, scalar1=lp1[:],
                                    scalar2=None, op0=Op.is_equal)
            selscr = pool.tile([BL, C], F32)
            selv = pool.tile([BL, 1], F32)
            nc.vector.tensor_tensor(out=selscr[:], in0=sg[:], in1=onehotF[:],
                                    op=Op.mult)
            nc.vector.tensor_reduce(out=selv[:], in_=selscr[:],
                                    axis=mybir.AxisListType.X, op=Op.add)

            pack = pool.tile([BL, 2], F32)
            nc.vector.tensor_tensor(out=pack[:, 0:1], in0=lse[:], in1=selv[:],
                                    op=Op.subtract)

            mx = pool.tile([BL, 8], F32)
            mi = pool.tile([BL, 8], U32)
            nc.vector.max_with_indices(out_max=mx[:], out_indices=mi[:],
                                       in_=sg[:])
            nc.vector.tensor_scalar(out=pack[:, 1:2], in0=mi[:, 0:1],
                                    scalar1=-1.0, scalar2=None, op0=Op.add)

            nc.sync.dma_start(out=out_d, in_=pack[:])

    nc.compile()
    return nc


_NC = None
_CONSTS = None


def _get_nc():
    global _NC
    if _NC is None:
        _NC = _build()
    return _NC


def _get_consts():
    global _CONSTS
    if _CONSTS is None:
        _CONSTS = _make_consts()
    return _CONSTS


def kernel(sequence_output, sot_positions, labels):
    seq = np.ascontiguousarray(np.asarray(sequence_output, dtype=np.float32))
    mask = np.asarray(sot_positions)
    lab = np.ascontiguousarray(np.asarray(labels, dtype=np.int32))
    maskf = np.ascontiguousarray(mask.astype(ml_dtypes.bfloat16))
    labp1 = np.ascontiguousarray(
        (lab.astype(np.float32) + 1.0).reshape(NCORES, BL, 1))
    ca, cb, ebf = _get_consts()

    in_maps = []
    for i in range(NCORES):
        r = slice(i * BL, (i + 1) * BL)
        in_maps.append({
            "seq": seq[r].reshape(BL * S, D),
            "maskf": maskf[r],
            "labp1": labp1[i],
            "ca": ca,
            "cb": cb,
            "ebf": ebf,
        })

    res = run_bass_kernel_spmd(_get_nc(), in_maps, core_ids=list(range(NCORES)))
    packs = np.stack([np.asarray(r["out"]) for r in res.results])  # [8, BL, 2]
    ce = packs[:, :, 0].reshape(-1)
    pred = np.rint(packs[:, :, 1]).astype(np.int32)

    valid = lab >= 0
    n_valid = np.float32(valid.sum())
    loss = np.float32(
        np.sum(np.where(valid, ce, np.float32(0.0)), dtype=np.float32)
        / max(n_valid, np.float32(1.0)))
    return loss, pred.reshape(-1), lab
